# revision 1
# baseline (speedup 1.0000x reference)
"""Trainium2 Bass kernel for CIN: out[b,m,d] = sigmoid(einsum('bid,bjd,ijm', x0, x, K)).

Shapes (hardcoded): x0,x [4096, 40, 64] f32, kernel [40, 40, 128] f32,
out [4096, 128, 64] f32.

Sharding: data-parallel over batch B across 8 NeuronCores (512 b each).

Per-core pipeline (groups of 8 b's; free dim = 8*64 = 512), all bf16 on
the engines, fp32 accumulation in PSUM:
  - The interaction tensor Z[(i j), (b d)] = x0[i,(b d)] * x[j,(b d)] is
    built directly in matmul-rhs layout ((i j) on partitions) -- no PE
    transposes.  ij-space is blocked 3 i-rows per 128-partition chunk:
    chunk c row p -> (i, j) = (3c + p//40, p%40), p < 120 (8 pad rows).
    14 chunks cover all 40 i's.  Then per chunk:
      * ONE replication matmul with constant 0/1 weights expands x0 rows
        into the chunk's i-pattern: psA[p, bd] = x0T[3c+p//40, bd] (PSUM)
      * the j-side operand B[p, bd] = xT[p%40, bd] is CHUNK-INDEPENDENT,
        host-packed, and DMA'd once per group into SBUF (bf16)
      * the DVE multiply zc = psA * B.  Chunks run in PAIRS sharing a
        2-bank PSUM tile; for 3 of 7 pairs the Scalar engine first
        evacuates psA to SBUF bf16 through its own PSUM port so the DVE
        multiply runs in 2x perf mode (both operands SBUF bf16); the
        rest multiply straight from PSUM at 1x (single DVE PSUM port)
      * one accumulated matmul  pso += K_c^T @ zc  (contraction (i j))
  - a short dense matmul spin at kernel start warms the PE HAM
    clock-gate to 2.4 GHz; steady-state gaps keep it warm
  - sigmoid fused into PSUM evacuation on ACT, DMA out.

Host-side prep (not on the HW critical path): inputs cast to bf16 and
packed so every DMA is a dense, partition-contiguous load.
"""

import sys

for _p in ("/opt/trn_rl_repo", "/root/.axon_site/_ro/trn_rl_repo"):
    if _p not in sys.path:
        sys.path.insert(0, _p)

from contextlib import ExitStack

import numpy as np
import ml_dtypes

import concourse.bass as bass
from concourse import bacc
import concourse.tile as tile
from concourse import mybir
from concourse.bass_utils import run_bass_kernel_spmd

B, F0, F, D, M = 4096, 40, 40, 64, 128
NCORES = 8
NB = B // NCORES            # 512 b per core
GB = 8                      # b's per group
FREE = GB * D               # 512 = matmul free dim = one PSUM bank (f32)
NG = NB // GB               # 64 groups per core
IJ = F0 * F                 # 1600
IPC = 3                     # i-rows per chunk
ROWS = IPC * F              # 120 valid rows per chunk
NCHUNK = (F0 + IPC - 1) // IPC  # 14

f32 = mybir.dt.float32
bf16 = mybir.dt.bfloat16
BF16 = ml_dtypes.bfloat16


def _pack_kernel(kernel_np: np.ndarray) -> np.ndarray:
    """K[i,j,m] -> kwT [128, NCHUNK, M] bf16,
    kwT[p, c, m] = K[3c + p//40, p%40, m] (zero where invalid)."""
    kf = np.zeros((NCHUNK, 128, M), dtype=np.float32)
    p = np.arange(ROWS)
    for c in range(NCHUNK):
        i = IPC * c + p // F
        valid = i < F0
        kf[c, p[valid]] = kernel_np[i[valid], p[valid] % F]
    return np.ascontiguousarray(kf.transpose(1, 0, 2).astype(BF16))


def _pack_reps() -> np.ndarray:
    """Constant replication weights [F0, NCHUNK, 128] bf16:
    rp[k, c, p] = (k == 3c + p//40), p < 120."""
    rp = np.zeros((F0, NCHUNK, 128), dtype=np.float32)
    p = np.arange(ROWS)
    for c in range(NCHUNK):
        i = IPC * c + p // F
        valid = i < F0
        rp[i[valid], c, p[valid]] = 1.0
    return np.ascontiguousarray(rp.astype(BF16))


def _pack_x(x0: np.ndarray) -> np.ndarray:
    """-> xp [NCORES, NG, F0, FREE] bf16: x0T per (core, group)."""
    x0r = x0.reshape(NCORES, NG, GB, F0, D).transpose(0, 1, 3, 2, 4)
    return np.ascontiguousarray(
        x0r.reshape(NCORES, NG, F0, FREE).astype(BF16))


def _pack_b(x: np.ndarray) -> np.ndarray:
    """-> bp [NCORES, NG, 128, FREE] bf16: B[p, bd] = xT[p%40, bd] for
    p < 120, zero pad rows."""
    xr = x.reshape(NCORES, NG, GB, F, D).transpose(0, 1, 3, 2, 4)
    xr = xr.reshape(NCORES, NG, F, FREE).astype(BF16)
    bp = np.zeros((NCORES, NG, 128, FREE), dtype=BF16)
    bp[:, :, 0:ROWS, :] = np.concatenate([xr] * IPC, axis=2)
    return bp


def _build(nb: int):
    ng = nb // GB

    nc = bacc.Bacc("TRN2", num_devices=8)
    xp = nc.declare_dram_parameter("xp", [ng, F0, FREE], bf16, isOutput=False)
    bpp = nc.declare_dram_parameter("bp", [ng, 128, FREE], bf16, isOutput=False)
    kp = nc.declare_dram_parameter("kp", [128, NCHUNK, M], bf16, isOutput=False)
    rep = nc.declare_dram_parameter("rep", [F0, NCHUNK, 128], bf16, isOutput=False)
    outp = nc.declare_dram_parameter("out", [nb, M, D], f32, isOutput=True)

    with ExitStack() as ctx:
        tc = ctx.enter_context(tile.TileContext(nc))
        singles = ctx.enter_context(tc.tile_pool(name="singles", bufs=1))
        xx_pool = ctx.enter_context(tc.tile_pool(name="xx", bufs=3))
        bb_pool = ctx.enter_context(tc.tile_pool(name="bb", bufs=3))
        zc_pool = ctx.enter_context(tc.tile_pool(name="zc", bufs=4))
        ea_pool = ctx.enter_context(tc.tile_pool(name="ea", bufs=3))
        osb_pool = ctx.enter_context(tc.tile_pool(name="osb", bufs=3))
        psa_pool = ctx.enter_context(tc.tile_pool(name="psa", bufs=3, space="PSUM"))
        pso_pool = ctx.enter_context(tc.tile_pool(name="pso", bufs=2, space="PSUM"))

        kw = singles.tile([128, NCHUNK, M], bf16)
        nc.sync.dma_start(out=kw, in_=kp[:])
        rp = singles.tile([F0, NCHUNK, 128], bf16)
        nc.sync.dma_start(out=rp, in_=rep[:])

        # HAM warm-up spin: ~16 dense back-to-back matmuls (~5us) raise the
        # PE clock-gate to K=8/8 (2.4 GHz) before the real work starts.  The
        # steady-state loop never leaves the PE idle >3.4us, so it stays
        # warm for the whole kernel.  Without this the PE runs at 1.2 GHz
        # throughout (per-chunk DVE waits prevent a 3.4us busy window).
        spin_w = singles.tile([128, 128], bf16)
        nc.vector.memset(spin_w, 0.0)
        spin_r = singles.tile([128, FREE], bf16)
        nc.vector.memset(spin_r, 0.0)
        ps_spin = pso_pool.tile([128, FREE], f32, tag="pso")
        for _ in range(10):
            nc.tensor.matmul(ps_spin, spin_w, spin_r, start=True, stop=True)

        for g in range(ng):
            bsl = slice(g * GB, (g + 1) * GB)
            xx = xx_pool.tile([F0, FREE], bf16, tag="xx")
            nc.sync.dma_start(out=xx, in_=xp[g])
            bb = bb_pool.tile([128, FREE], bf16, tag="bb")
            nc.sync.dma_start(out=bb, in_=bpp[g])

            pso = pso_pool.tile([128, FREE], f32, tag="pso")
            # chunks in pairs: both rep outputs land in one 2-bank PSUM
            # tile so a single DVE multiply (FD=1024) covers 2 chunks --
            # amortizes the DVE per-instruction + PSUM-port overhead.
            for q in range(NCHUNK // 2):
                psA = psa_pool.tile([128, 2, FREE], f32, tag="psa")
                nc.tensor.matmul(psA[:, 0, :], rp[:, 2 * q, :], xx,
                                 start=True, stop=True)
                nc.tensor.matmul(psA[:, 1, :], rp[:, 2 * q + 1, :], xx,
                                 start=True, stop=True)
                zc = zc_pool.tile([128, 2, FREE], bf16, tag="zc")
                if q in (1, 3, 5):
                    # ACT pre-evacuates psA to SBUF bf16 through its own
                    # PSUM port; the DVE multiplies then run in 2x mode
                    # (both operands SBUF bf16, no broadcast).
                    ea = ea_pool.tile([128, 2, FREE], bf16, tag="ea")
                    nc.scalar.copy(out=ea, in_=psA)
                    nc.vector.tensor_tensor(out=zc[:, 0, :], in0=ea[:, 0, :],
                                            in1=bb, op=mybir.AluOpType.mult)
                    nc.vector.tensor_tensor(out=zc[:, 1, :], in0=ea[:, 1, :],
                                            in1=bb, op=mybir.AluOpType.mult)
                else:
                    nc.vector.tensor_tensor(
                        out=zc, in0=psA,
                        in1=bb.unsqueeze(1).broadcast_to((128, 2, FREE)),
                        op=mybir.AluOpType.mult)
                nc.tensor.matmul(pso, kw[:, 2 * q, :], zc[:, 0, :],
                                 start=(q == 0), stop=False)
                nc.tensor.matmul(pso, kw[:, 2 * q + 1, :], zc[:, 1, :],
                                 start=False, stop=(q == NCHUNK // 2 - 1))

            osb = osb_pool.tile([128, GB, D], f32, tag="osb")
            nc.scalar.activation(osb.rearrange("m b d -> m (b d)"), pso,
                                 mybir.ActivationFunctionType.Sigmoid)
            nc.sync.dma_start(out=outp[bsl].transpose([1, 0, 2]), in_=osb)

    nc.finalize()
    return nc


_NC_CACHE = {}


def _get_nc():
    if "nc" not in _NC_CACHE:
        _NC_CACHE["nc"] = _build(NB)
    return _NC_CACHE["nc"]


def _make_in_maps(x0: np.ndarray, x: np.ndarray, kernel: np.ndarray):
    x0 = np.ascontiguousarray(np.asarray(x0, dtype=np.float32))
    x = np.ascontiguousarray(np.asarray(x, dtype=np.float32))
    kw = _pack_kernel(np.asarray(kernel, dtype=np.float32))
    rp = _pack_reps()
    xp = _pack_x(x0)
    bp = _pack_b(x)
    return [
        {"xp": xp[i], "bp": bp[i], "kp": kw, "rep": rp}
        for i in range(NCORES)
    ]


def kernel(x0: np.ndarray, x: np.ndarray, kernel: np.ndarray) -> np.ndarray:
    nc = _get_nc()
    in_maps = _make_in_maps(x0, x, kernel)
    res = run_bass_kernel_spmd(nc, in_maps, list(range(NCORES)))
    out = np.concatenate([np.asarray(r["out"]) for r in res.results], axis=0)
    return out.astype(np.float32)



# revision 9
# speedup vs baseline: 2.1895x; 2.1895x over previous
"""Trainium2 Bass kernel for CIN: out[b,m,d] = sigmoid(einsum('bid,bjd,ijm', x0, x, K)).

Shapes (hardcoded): x0,x [4096, 40, 64] f32, kernel [40, 40, 128] f32,
out [4096, 128, 64] f32.

Sharding: data-parallel over batch B across 8 NeuronCores (512 b each).

Per-core pipeline (groups of 8 b's; free dim = 8*64 = 512), bf16 on the
engines, fp32 accumulation in PSUM.  The interaction tensor
Z[(i j), (b d)] = x0[i,(b d)] * x[j,(b d)] is built with (i j) on
partitions, blocked 3 i-rows per 128-partition chunk (14 chunks):

  - zin[p, c, bd] = x0T[3c + p//40, bd] (the replicated-x0 "A side"):
      * chunks 0..7  are HOST-replicated and DMA'd straight into zin
        (pure layout transform; DMA has headroom, PE/ACT do not)
      * chunks 8..13 come from 6 replication matmuls with constant 0/1
        weights (PSUM), evacuated to zin by the Scalar engine in 3
        pair-copies (PSUM -> SBUF bf16)
  - ONE giant DVE multiply per group builds all 14 chunks of
    zc = zin * bb:  in1 = bb[128, 512] broadcast over the chunk axis.
    All operands SBUF bf16 with unit inner stride -> DVE 2x mode
    (measured 3.88us for FD=7168).
  - 14 accumulated matmuls  pso += K_c^T @ zc_c  (contraction (i j))
  - sigmoid fused into PSUM evacuation on ACT -> bf16, DMA out as
    [M, b, d]; host transposes back to [b, M, d] and widens to f32.

Issue order is software-pipelined with lookahead 2 (reps for group g+2
issue before mains of group g) so the PE never sits behind the
DVE/ACT/DMA chain of the current group.

Host-side prep (not on the HW critical path): inputs cast to bf16 and
packed so every DMA is a dense, partition-contiguous load.
"""

import sys

for _p in ("/opt/trn_rl_repo", "/root/.axon_site/_ro/trn_rl_repo"):
    if _p not in sys.path:
        sys.path.insert(0, _p)

from contextlib import ExitStack

import numpy as np
import ml_dtypes

import concourse.bass as bass
from concourse import bacc
import concourse.tile as tile
from concourse import mybir
from concourse.bass_utils import run_bass_kernel_spmd

B, F0, F, D, M = 4096, 40, 40, 64, 128
NCORES = 8
NB = B // NCORES            # 512 b per core
GB = 8                      # b's per group
FREE = GB * D               # 512 = matmul free dim = one PSUM bank (f32)
NG = NB // GB               # 64 groups per core
IPC = 3                     # i-rows per chunk
ROWS = IPC * F              # 120 valid rows per chunk
NCHUNK = (F0 + IPC - 1) // IPC  # 14
HOSTC = 8                   # chunks 0..HOSTC-1 replicated on the host
PEC = NCHUNK - HOSTC        # 6 chunks replicated on the PE (3 pairs)
PACK_REPS = False           # tile_position row-packing of rep pairs
LOOKAHEAD = 2

f32 = mybir.dt.float32
bf16 = mybir.dt.bfloat16
BF16 = ml_dtypes.bfloat16


def _pack_kernel(kernel_np: np.ndarray) -> np.ndarray:
    """K[i,j,m] -> kwT [128, NCHUNK, M] bf16,
    kwT[p, c, m] = K[3c + p//40, p%40, m] (zero where invalid)."""
    kf = np.zeros((NCHUNK, 128, M), dtype=np.float32)
    p = np.arange(ROWS)
    for c in range(NCHUNK):
        i = IPC * c + p // F
        valid = i < F0
        kf[c, p[valid]] = kernel_np[i[valid], p[valid] % F]
    return np.ascontiguousarray(kf.transpose(1, 0, 2).astype(BF16))


def _pack_reps() -> np.ndarray:
    """Constant replication weights [104, PEC//2, 2, 128] bf16 for the PE
    chunks (HOSTC..NCHUNK-1).  Slot [0:40, q, s] holds the weights for
    chunk HOSTC+2q+s (base-0 operands, unpacked mode); slot
    [64:104, q, 1] duplicates the odd chunk's weights so a packed pair
    can run as row-tiles (0,0) and (64,0)."""
    rp = np.zeros((104, PEC // 2, 2, 128), dtype=np.float32)
    p = np.arange(ROWS)
    for q in range(PEC // 2):
        for s in (0, 1):
            c = HOSTC + 2 * q + s
            i = IPC * c + p // F
            valid = i < F0
            rp[i[valid], q, s, p[valid]] = 1.0
            if s == 1:
                rp[64 + i[valid], q, s, p[valid]] = 1.0
    return np.ascontiguousarray(rp.astype(BF16))


def _pack_x(x0: np.ndarray) -> np.ndarray:
    """-> xp [NCORES, NG, 2, F0, FREE] bf16: x0T per (core, group),
    duplicated so a copy can sit at partitions 64:104 for packed reps."""
    x0r = x0.reshape(NCORES, NG, GB, F0, D).transpose(0, 1, 3, 2, 4)
    x0r = x0r.reshape(NCORES, NG, F0, FREE).astype(BF16)
    return np.ascontiguousarray(
        np.broadcast_to(x0r[:, :, None], (NCORES, NG, 2, F0, FREE)))


def _pack_b(x: np.ndarray) -> np.ndarray:
    """-> bp [NCORES, NG, 128, FREE] bf16: B[p, bd] = xT[p%40, bd] for
    p < 120, zero pad rows."""
    xr = x.reshape(NCORES, NG, GB, F, D).transpose(0, 1, 3, 2, 4)
    xr = xr.reshape(NCORES, NG, F, FREE).astype(BF16)
    bp = np.zeros((NCORES, NG, 128, FREE), dtype=BF16)
    bp[:, :, 0:ROWS, :] = np.concatenate([xr] * IPC, axis=2)
    return bp


def _pack_hostrep(x0: np.ndarray) -> np.ndarray:
    """-> hp [NCORES, NG, 128, HOSTC, FREE] bf16:
    hp[.., p, c, bd] = x0T[3c + p//40, bd] for p < 120, zero pad rows."""
    x0r = x0.reshape(NCORES, NG, GB, F0, D).transpose(0, 1, 3, 2, 4)
    x0r = np.ascontiguousarray(x0r.reshape(NCORES, NG, F0, FREE)).astype(BF16)
    hp = np.zeros((NCORES, NG, 128, HOSTC, FREE), dtype=BF16)
    p = np.arange(ROWS)
    for c in range(HOSTC):
        hp[:, :, 0:ROWS, c, :] = x0r[:, :, IPC * c + p // F, :]
    return hp


def _build(nb: int):
    ng = nb // GB

    nc = bacc.Bacc("TRN2", num_devices=8)
    xp = nc.declare_dram_parameter("xp", [ng, 2, F0, FREE], bf16, isOutput=False)
    bpp = nc.declare_dram_parameter("bp", [ng, 128, FREE], bf16, isOutput=False)
    hpp = nc.declare_dram_parameter("hp", [ng, 128, HOSTC, FREE], bf16,
                                    isOutput=False)
    kp = nc.declare_dram_parameter("kp", [128, NCHUNK, M], bf16, isOutput=False)
    rep = nc.declare_dram_parameter("rep", [104, PEC // 2, 2, 128], bf16,
                                    isOutput=False)
    outp = nc.declare_dram_parameter("out", [M, nb, D], bf16, isOutput=True)

    with ExitStack() as ctx:
        tc = ctx.enter_context(tile.TileContext(nc))
        singles = ctx.enter_context(tc.tile_pool(name="singles", bufs=1))
        xx_pool = ctx.enter_context(tc.tile_pool(name="xx", bufs=4))
        bb_pool = ctx.enter_context(tc.tile_pool(name="bb", bufs=4))
        zin_pool = ctx.enter_context(tc.tile_pool(name="zin", bufs=4))
        zc_pool = ctx.enter_context(tc.tile_pool(name="zc", bufs=3))
        osb_pool = ctx.enter_context(tc.tile_pool(name="osb", bufs=3))
        psa_pool = ctx.enter_context(tc.tile_pool(name="psa", bufs=3, space="PSUM"))
        pso_pool = ctx.enter_context(tc.tile_pool(name="pso", bufs=2, space="PSUM"))

        kw = singles.tile([128, NCHUNK, M], bf16)
        nc.sync.dma_start(out=kw, in_=kp[:])
        rp = singles.tile([104, PEC // 2, 2, 128], bf16)
        nc.sync.dma_start(out=rp, in_=rep[:])

        # HAM warm-up spin: dense back-to-back matmuls raise the PE
        # clock-gate toward 2.4 GHz while the first groups' DMAs land.
        spin_w = singles.tile([128, 128], bf16)
        nc.vector.memset(spin_w, 0.0)
        spin_r = singles.tile([128, FREE], bf16)
        nc.vector.memset(spin_r, 0.0)
        ps_spin = pso_pool.tile([128, FREE], f32, tag="pso")
        for _ in range(12):
            nc.tensor.matmul(ps_spin, spin_w, spin_r, start=True, stop=True)

        xxs = [None] * ng
        bbs = [None] * ng
        zins = [None] * ng

        def load(g):
            xx = xx_pool.tile([128, FREE], bf16, tag="xx")
            nc.sync.dma_start(out=xx[0:F0, :], in_=xp[g, 0])
            if PACK_REPS:
                nc.sync.dma_start(out=xx[64:64 + F0, :], in_=xp[g, 1])
            bb = bb_pool.tile([128, FREE], bf16, tag="bb")
            nc.sync.dma_start(out=bb, in_=bpp[g])
            zin = zin_pool.tile([128, NCHUNK, FREE], bf16, tag="zin")
            nc.sync.dma_start(out=zin[:, 0:HOSTC, :], in_=hpp[g])
            xxs[g], bbs[g], zins[g] = xx, bb, zin

        def reps(g):
            xx, zin = xxs[g], zins[g]
            for q in range(PEC // 2):
                psa = psa_pool.tile([128, 2, FREE], f32, tag="psa")
                nc.tensor.matmul(psa[:, 0, :], rp[0:F0, q, 0, :],
                                 xx[0:F0, :], start=True, stop=True)
                if PACK_REPS:
                    nc.tensor.matmul(psa[:, 1, :], rp[64:64 + F0, q, 1, :],
                                     xx[64:64 + F0, :], start=True, stop=True)
                else:
                    nc.tensor.matmul(psa[:, 1, :], rp[0:F0, q, 1, :],
                                     xx[0:F0, :], start=True, stop=True)
                c = HOSTC + 2 * q
                nc.scalar.copy(out=zin[:, c:c + 2, :], in_=psa)

        def mult(g):
            zc = zc_pool.tile([128, NCHUNK, FREE], bf16, tag="zc")
            nc.vector.tensor_tensor(
                out=zc, in0=zins[g],
                in1=bbs[g].unsqueeze(1).broadcast_to((128, NCHUNK, FREE)),
                op=mybir.AluOpType.mult)
            return zc

        zcs = [None] * ng

        def mains(g):
            zc = zcs[g]
            pso = pso_pool.tile([128, FREE], f32, tag="pso")
            for c in range(NCHUNK):
                nc.tensor.matmul(pso, kw[:, c, :], zc[:, c, :],
                                 start=(c == 0), stop=(c == NCHUNK - 1))
            osb = osb_pool.tile([128, GB, D], bf16, tag="osb")
            nc.scalar.activation(osb.rearrange("m b d -> m (b d)"), pso,
                                 mybir.ActivationFunctionType.Sigmoid)
            nc.sync.dma_start(out=outp[:, g * GB:(g + 1) * GB, :], in_=osb)

        for g in range(min(LOOKAHEAD + 1, ng)):
            load(g)
        for g in range(min(LOOKAHEAD, ng)):
            reps(g)
        for g in range(ng):
            if g + LOOKAHEAD + 1 < ng:
                load(g + LOOKAHEAD + 1)
            if g + LOOKAHEAD < ng:
                reps(g + LOOKAHEAD)
            zcs[g] = mult(g)
            mains(g)

    nc.finalize()
    return nc


_NC_CACHE = {}


def _get_nc():
    if "nc" not in _NC_CACHE:
        _NC_CACHE["nc"] = _build(NB)
    return _NC_CACHE["nc"]


def _make_in_maps(x0: np.ndarray, x: np.ndarray, kernel: np.ndarray):
    x0 = np.ascontiguousarray(np.asarray(x0, dtype=np.float32))
    x = np.ascontiguousarray(np.asarray(x, dtype=np.float32))
    kw = _pack_kernel(np.asarray(kernel, dtype=np.float32))
    rp = _pack_reps()
    xp = _pack_x(x0)
    bp = _pack_b(x)
    hp = _pack_hostrep(x0)
    return [
        {"xp": xp[i], "bp": bp[i], "hp": hp[i], "kp": kw, "rep": rp}
        for i in range(NCORES)
    ]


def kernel(x0: np.ndarray, x: np.ndarray, kernel: np.ndarray) -> np.ndarray:
    nc = _get_nc()
    in_maps = _make_in_maps(x0, x, kernel)
    res = run_bass_kernel_spmd(nc, in_maps, list(range(NCORES)))
    outs = [
        np.asarray(r["out"]).astype(np.float32).transpose(1, 0, 2)
        for r in res.results
    ]
    return np.ascontiguousarray(np.concatenate(outs, axis=0))


# revision 10
# speedup vs baseline: 2.3743x; 1.0844x over previous
"""Trainium2 Bass kernel for CIN: out[b,m,d] = sigmoid(einsum('bid,bjd,ijm', x0, x, K)).

Shapes (hardcoded): x0,x [4096, 40, 64] f32, kernel [40, 40, 128] f32,
out [4096, 128, 64] f32.

Sharding: data-parallel over batch B across 8 NeuronCores (512 b each).

Per-core pipeline (groups of 8 b's; free dim = 8*64 = 512), bf16 on the
engines, fp32 accumulation in PSUM.  The interaction tensor
Z[(i j), (b d)] = x0[i,(b d)] * x[j,(b d)] is built with (i j) on
partitions, blocked 3 i-rows per 128-partition chunk (14 chunks):

  - zin[p, c, bd] = x0T[3c + p//40, bd] (the replicated-x0 "A side"):
      * chunks 0..7  are HOST-replicated and DMA'd straight into zin
        (pure layout transform; DMA has headroom, PE/ACT do not)
      * chunks 8..13 come from 6 replication matmuls with constant 0/1
        weights (PSUM), evacuated to zin by the Scalar engine in 3
        pair-copies (PSUM -> SBUF bf16)
  - ONE giant DVE multiply per group builds all 14 chunks of
    zc = zin * bb:  in1 = bb[128, 512] broadcast over the chunk axis.
    All operands SBUF bf16 with unit inner stride -> DVE 2x mode
    (measured 3.88us for FD=7168).
  - 14 accumulated matmuls  pso += K_c^T @ zc_c  (contraction (i j))
  - sigmoid fused into PSUM evacuation on ACT -> bf16, DMA out as
    [M, b, d]; host transposes back to [b, M, d] and widens to f32.

Issue order is software-pipelined with lookahead 2 (reps for group g+2
issue before mains of group g) so the PE never sits behind the
DVE/ACT/DMA chain of the current group.

Host-side prep (not on the HW critical path): inputs cast to bf16 and
packed so every DMA is a dense, partition-contiguous load.
"""

import sys

for _p in ("/opt/trn_rl_repo", "/root/.axon_site/_ro/trn_rl_repo"):
    if _p not in sys.path:
        sys.path.insert(0, _p)

from contextlib import ExitStack

import numpy as np
import ml_dtypes

import concourse.bass as bass
from concourse import bacc
import concourse.tile as tile
from concourse import mybir
from concourse.bass_utils import run_bass_kernel_spmd

B, F0, F, D, M = 4096, 40, 40, 64, 128
NCORES = 8
NB = B // NCORES            # 512 b per core
GB = 8                      # b's per group
FREE = GB * D               # 512 = matmul free dim = one PSUM bank (f32)
NG = NB // GB               # 64 groups per core
IPC = 3                     # i-rows per chunk
ROWS = IPC * F              # 120 valid rows per chunk
NCHUNK = (F0 + IPC - 1) // IPC  # 14
HOSTC = 8                   # chunks 0..HOSTC-1 replicated on the host
PEC = NCHUNK - HOSTC        # 6 chunks replicated on the PE (3 pairs)
PACK_REPS = True            # tile_position row-packing of rep pairs
LOOKAHEAD = 2

f32 = mybir.dt.float32
bf16 = mybir.dt.bfloat16
BF16 = ml_dtypes.bfloat16


def _pack_kernel(kernel_np: np.ndarray) -> np.ndarray:
    """K[i,j,m] -> kwT [128, NCHUNK, M] bf16,
    kwT[p, c, m] = K[3c + p//40, p%40, m] (zero where invalid)."""
    kf = np.zeros((NCHUNK, 128, M), dtype=np.float32)
    p = np.arange(ROWS)
    for c in range(NCHUNK):
        i = IPC * c + p // F
        valid = i < F0
        kf[c, p[valid]] = kernel_np[i[valid], p[valid] % F]
    return np.ascontiguousarray(kf.transpose(1, 0, 2).astype(BF16))


def _pack_reps() -> np.ndarray:
    """Constant replication weights [104, PEC//2, 2, 128] bf16 for the PE
    chunks (HOSTC..NCHUNK-1).  Slot [0:40, q, s] holds the weights for
    chunk HOSTC+2q+s (base-0 operands, unpacked mode); slot
    [64:104, q, 1] duplicates the odd chunk's weights so a packed pair
    can run as row-tiles (0,0) and (64,0)."""
    rp = np.zeros((104, PEC // 2, 2, 128), dtype=np.float32)
    p = np.arange(ROWS)
    for q in range(PEC // 2):
        for s in (0, 1):
            c = HOSTC + 2 * q + s
            i = IPC * c + p // F
            valid = i < F0
            rp[i[valid], q, s, p[valid]] = 1.0
            if s == 1:
                rp[64 + i[valid], q, s, p[valid]] = 1.0
    return np.ascontiguousarray(rp.astype(BF16))


def _pack_x(x0: np.ndarray) -> np.ndarray:
    """-> xp [NCORES, NG, 2, F0, FREE] bf16: x0T per (core, group),
    duplicated so a copy can sit at partitions 64:104 for packed reps."""
    x0r = x0.reshape(NCORES, NG, GB, F0, D).transpose(0, 1, 3, 2, 4)
    x0r = x0r.reshape(NCORES, NG, F0, FREE).astype(BF16)
    return np.ascontiguousarray(
        np.broadcast_to(x0r[:, :, None], (NCORES, NG, 2, F0, FREE)))


def _pack_b(x: np.ndarray) -> np.ndarray:
    """-> bp [NCORES, NG, 128, FREE] bf16: B[p, bd] = xT[p%40, bd] for
    p < 120, zero pad rows."""
    xr = x.reshape(NCORES, NG, GB, F, D).transpose(0, 1, 3, 2, 4)
    xr = xr.reshape(NCORES, NG, F, FREE).astype(BF16)
    bp = np.zeros((NCORES, NG, 128, FREE), dtype=BF16)
    bp[:, :, 0:ROWS, :] = np.concatenate([xr] * IPC, axis=2)
    return bp


def _pack_hostrep(x0: np.ndarray) -> np.ndarray:
    """-> hp [NCORES, NG, 128, HOSTC, FREE] bf16:
    hp[.., p, c, bd] = x0T[3c + p//40, bd] for p < 120, zero pad rows."""
    x0r = x0.reshape(NCORES, NG, GB, F0, D).transpose(0, 1, 3, 2, 4)
    x0r = np.ascontiguousarray(x0r.reshape(NCORES, NG, F0, FREE)).astype(BF16)
    hp = np.zeros((NCORES, NG, 128, HOSTC, FREE), dtype=BF16)
    p = np.arange(ROWS)
    for c in range(HOSTC):
        hp[:, :, 0:ROWS, c, :] = x0r[:, :, IPC * c + p // F, :]
    return hp


def _build(nb: int):
    ng = nb // GB

    nc = bacc.Bacc("TRN2", num_devices=8)
    xp = nc.declare_dram_parameter("xp", [ng, 2, F0, FREE], bf16, isOutput=False)
    bpp = nc.declare_dram_parameter("bp", [ng, 128, FREE], bf16, isOutput=False)
    hpp = nc.declare_dram_parameter("hp", [ng, 128, HOSTC, FREE], bf16,
                                    isOutput=False)
    kp = nc.declare_dram_parameter("kp", [128, NCHUNK, M], bf16, isOutput=False)
    rep = nc.declare_dram_parameter("rep", [104, PEC // 2, 2, 128], bf16,
                                    isOutput=False)
    outp = nc.declare_dram_parameter("out", [M, nb, D], bf16, isOutput=True)

    with ExitStack() as ctx:
        tc = ctx.enter_context(tile.TileContext(nc))
        singles = ctx.enter_context(tc.tile_pool(name="singles", bufs=1))
        xx_pool = ctx.enter_context(tc.tile_pool(name="xx", bufs=4))
        bb_pool = ctx.enter_context(tc.tile_pool(name="bb", bufs=4))
        zin_pool = ctx.enter_context(tc.tile_pool(name="zin", bufs=4))
        zc_pool = ctx.enter_context(tc.tile_pool(name="zc", bufs=3))
        osb_pool = ctx.enter_context(tc.tile_pool(name="osb", bufs=3))
        psa_pool = ctx.enter_context(tc.tile_pool(name="psa", bufs=3, space="PSUM"))
        pso_pool = ctx.enter_context(tc.tile_pool(name="pso", bufs=2, space="PSUM"))

        kw = singles.tile([128, NCHUNK, M], bf16)
        nc.sync.dma_start(out=kw, in_=kp[:])
        rp = singles.tile([104, PEC // 2, 2, 128], bf16)
        nc.sync.dma_start(out=rp, in_=rep[:])

        # HAM warm-up spin: dense back-to-back matmuls raise the PE
        # clock-gate toward 2.4 GHz while the first groups' DMAs land.
        spin_w = singles.tile([128, 128], bf16)
        nc.vector.memset(spin_w, 0.0)
        spin_r = singles.tile([128, FREE], bf16)
        nc.vector.memset(spin_r, 0.0)
        ps_spin = pso_pool.tile([128, FREE], f32, tag="pso")
        for _ in range(12):
            nc.tensor.matmul(ps_spin, spin_w, spin_r, start=True, stop=True)

        xxs = [None] * ng
        bbs = [None] * ng
        zins = [None] * ng

        def load(g):
            xx = xx_pool.tile([128, FREE], bf16, tag="xx")
            nc.sync.dma_start(out=xx[0:F0, :], in_=xp[g, 0])
            if PACK_REPS:
                nc.sync.dma_start(out=xx[64:64 + F0, :], in_=xp[g, 1])
            bb = bb_pool.tile([128, FREE], bf16, tag="bb")
            nc.sync.dma_start(out=bb, in_=bpp[g])
            zin = zin_pool.tile([128, NCHUNK, FREE], bf16, tag="zin")
            nc.sync.dma_start(out=zin[:, 0:HOSTC, :], in_=hpp[g])
            xxs[g], bbs[g], zins[g] = xx, bb, zin

        def reps(g):
            xx, zin = xxs[g], zins[g]
            for q in range(PEC // 2):
                psa = psa_pool.tile([128, 2, FREE], f32, tag="psa")
                nc.tensor.matmul(psa[:, 0, :], rp[0:F0, q, 0, :],
                                 xx[0:F0, :], start=True, stop=True)
                if PACK_REPS:
                    nc.tensor.matmul(psa[:, 1, :], rp[64:64 + F0, q, 1, :],
                                     xx[64:64 + F0, :], start=True, stop=True)
                else:
                    nc.tensor.matmul(psa[:, 1, :], rp[0:F0, q, 1, :],
                                     xx[0:F0, :], start=True, stop=True)
                c = HOSTC + 2 * q
                nc.scalar.copy(out=zin[:, c:c + 2, :], in_=psa)

        def mult(g):
            zc = zc_pool.tile([128, NCHUNK, FREE], bf16, tag="zc")
            nc.vector.tensor_tensor(
                out=zc, in0=zins[g],
                in1=bbs[g].unsqueeze(1).broadcast_to((128, NCHUNK, FREE)),
                op=mybir.AluOpType.mult)
            return zc

        zcs = [None] * ng

        def mains(g):
            zc = zcs[g]
            pso = pso_pool.tile([128, FREE], f32, tag="pso")
            for c in range(NCHUNK):
                nc.tensor.matmul(pso, kw[:, c, :], zc[:, c, :],
                                 start=(c == 0), stop=(c == NCHUNK - 1))
            osb = osb_pool.tile([128, GB, D], bf16, tag="osb")
            nc.scalar.activation(osb.rearrange("m b d -> m (b d)"), pso,
                                 mybir.ActivationFunctionType.Sigmoid)
            nc.sync.dma_start(out=outp[:, g * GB:(g + 1) * GB, :], in_=osb)

        for g in range(min(LOOKAHEAD + 1, ng)):
            load(g)
        for g in range(min(LOOKAHEAD, ng)):
            reps(g)
        for g in range(ng):
            if g + LOOKAHEAD + 1 < ng:
                load(g + LOOKAHEAD + 1)
            if g + LOOKAHEAD < ng:
                reps(g + LOOKAHEAD)
            zcs[g] = mult(g)
            mains(g)

    nc.finalize()
    return nc


_NC_CACHE = {}


def _get_nc():
    if "nc" not in _NC_CACHE:
        _NC_CACHE["nc"] = _build(NB)
    return _NC_CACHE["nc"]


def _make_in_maps(x0: np.ndarray, x: np.ndarray, kernel: np.ndarray):
    x0 = np.ascontiguousarray(np.asarray(x0, dtype=np.float32))
    x = np.ascontiguousarray(np.asarray(x, dtype=np.float32))
    kw = _pack_kernel(np.asarray(kernel, dtype=np.float32))
    rp = _pack_reps()
    xp = _pack_x(x0)
    bp = _pack_b(x)
    hp = _pack_hostrep(x0)
    return [
        {"xp": xp[i], "bp": bp[i], "hp": hp[i], "kp": kw, "rep": rp}
        for i in range(NCORES)
    ]


def kernel(x0: np.ndarray, x: np.ndarray, kernel: np.ndarray) -> np.ndarray:
    nc = _get_nc()
    in_maps = _make_in_maps(x0, x, kernel)
    res = run_bass_kernel_spmd(nc, in_maps, list(range(NCORES)))
    outs = [
        np.asarray(r["out"]).astype(np.float32).transpose(1, 0, 2)
        for r in res.results
    ]
    return np.ascontiguousarray(np.concatenate(outs, axis=0))


# revision 12
# speedup vs baseline: 2.5590x; 1.0778x over previous
"""Trainium2 Bass kernel for CIN: out[b,m,d] = sigmoid(einsum('bid,bjd,ijm', x0, x, K)).

Shapes (hardcoded): x0,x [4096, 40, 64] f32, kernel [40, 40, 128] f32,
out [4096, 128, 64] f32.

Sharding: data-parallel over batch B across 8 NeuronCores (512 b each).

Per-core pipeline (groups of 8 b's; free dim = 8*64 = 512), bf16 on the
engines, fp32 accumulation in PSUM.  The interaction tensor
Z[(i j), (b d)] = x0[i,(b d)] * x[j,(b d)] is built with (i j) on
partitions, blocked 3 i-rows per 128-partition chunk (14 chunks):

  - zin[p, c, bd] = x0T[3c + p//40, bd] (the replicated-x0 "A side"):
      * chunks 0..7  are HOST-replicated and DMA'd straight into zin
        (pure layout transform; DMA has headroom, PE/ACT do not)
      * chunks 8..13 come from 6 replication matmuls with constant 0/1
        weights (PSUM), evacuated to zin by the Scalar engine in 3
        pair-copies (PSUM -> SBUF bf16)
  - ONE giant DVE multiply per group builds all 14 chunks of
    zc = zin * bb:  in1 = bb[128, 512] broadcast over the chunk axis.
    All operands SBUF bf16 with unit inner stride -> DVE 2x mode
    (measured 3.88us for FD=7168).
  - 14 accumulated matmuls  pso += K_c^T @ zc_c  (contraction (i j))
  - sigmoid fused into PSUM evacuation on ACT -> bf16, DMA out as
    [M, b, d]; host transposes back to [b, M, d] and widens to f32.

Issue order is software-pipelined with lookahead 2 (reps for group g+2
issue before mains of group g) so the PE never sits behind the
DVE/ACT/DMA chain of the current group.

Host-side prep (not on the HW critical path): inputs cast to bf16 and
packed so every DMA is a dense, partition-contiguous load.
"""

import sys

for _p in ("/opt/trn_rl_repo", "/root/.axon_site/_ro/trn_rl_repo"):
    if _p not in sys.path:
        sys.path.insert(0, _p)

from contextlib import ExitStack

import numpy as np
import ml_dtypes

import concourse.bass as bass
from concourse import bacc
import concourse.tile as tile
from concourse import mybir
from concourse.bass_utils import run_bass_kernel_spmd

B, F0, F, D, M = 4096, 40, 40, 64, 128
NCORES = 8
NB = B // NCORES            # 512 b per core
GB = 8                      # b's per group
FREE = GB * D               # 512 = matmul free dim = one PSUM bank (f32)
NG = NB // GB               # 64 groups per core
IPC = 3                     # i-rows per chunk
ROWS = IPC * F              # 120 valid rows per chunk
NCHUNK = (F0 + IPC - 1) // IPC  # 14
HOSTC = 8                   # chunks 0..HOSTC-1 replicated on the host
PEC = NCHUNK - HOSTC        # 6 chunks replicated on the PE (3 pairs)
PACK_REPS = True            # tile_position row-packing of rep pairs
LOOKAHEAD = 2

f32 = mybir.dt.float32
bf16 = mybir.dt.bfloat16
BF16 = ml_dtypes.bfloat16


def _pack_kernel(kernel_np: np.ndarray) -> np.ndarray:
    """K[i,j,m] -> kwT [128, NCHUNK, M] bf16,
    kwT[p, c, m] = K[3c + p//40, p%40, m] (zero where invalid)."""
    kf = np.zeros((NCHUNK, 128, M), dtype=np.float32)
    p = np.arange(ROWS)
    for c in range(NCHUNK):
        i = IPC * c + p // F
        valid = i < F0
        kf[c, p[valid]] = kernel_np[i[valid], p[valid] % F]
    return np.ascontiguousarray(kf.transpose(1, 0, 2).astype(BF16))


def _pack_reps() -> np.ndarray:
    """Constant replication weights [104, PEC//2, 2, 128] bf16 for the PE
    chunks (HOSTC..NCHUNK-1).  Slot [0:40, q, s] holds the weights for
    chunk HOSTC+2q+s (base-0 operands, unpacked mode); slot
    [64:104, q, 1] duplicates the odd chunk's weights so a packed pair
    can run as row-tiles (0,0) and (64,0)."""
    rp = np.zeros((104, PEC // 2, 2, 128), dtype=np.float32)
    p = np.arange(ROWS)
    for q in range(PEC // 2):
        for s in (0, 1):
            c = HOSTC + 2 * q + s
            i = IPC * c + p // F
            valid = i < F0
            rp[i[valid], q, s, p[valid]] = 1.0
            if s == 1:
                rp[64 + i[valid], q, s, p[valid]] = 1.0
    return np.ascontiguousarray(rp.astype(BF16))


def _pack_x(x0: np.ndarray) -> np.ndarray:
    """-> xp [NCORES, NG, 2, F0, FREE] bf16: x0T per (core, group),
    duplicated so a copy can sit at partitions 64:104 for packed reps."""
    x0r = x0.reshape(NCORES, NG, GB, F0, D).transpose(0, 1, 3, 2, 4)
    x0r = x0r.reshape(NCORES, NG, F0, FREE).astype(BF16)
    return np.ascontiguousarray(
        np.broadcast_to(x0r[:, :, None], (NCORES, NG, 2, F0, FREE)))


def _pack_b(x: np.ndarray) -> np.ndarray:
    """-> bp [NCORES, NG, 128, FREE] bf16: B[p, bd] = xT[p%40, bd] for
    p < 120, zero pad rows."""
    xr = x.reshape(NCORES, NG, GB, F, D).transpose(0, 1, 3, 2, 4)
    xr = xr.reshape(NCORES, NG, F, FREE).astype(BF16)
    bp = np.zeros((NCORES, NG, 128, FREE), dtype=BF16)
    bp[:, :, 0:ROWS, :] = np.concatenate([xr] * IPC, axis=2)
    return bp


def _pack_hostrep(x0: np.ndarray) -> np.ndarray:
    """-> hp [NCORES, NG, 128, HOSTC, FREE] bf16:
    hp[.., p, c, bd] = x0T[3c + p//40, bd] for p < 120, zero pad rows."""
    x0r = x0.reshape(NCORES, NG, GB, F0, D).transpose(0, 1, 3, 2, 4)
    x0r = np.ascontiguousarray(x0r.reshape(NCORES, NG, F0, FREE)).astype(BF16)
    hp = np.zeros((NCORES, NG, 128, HOSTC, FREE), dtype=BF16)
    p = np.arange(ROWS)
    for c in range(HOSTC):
        hp[:, :, 0:ROWS, c, :] = x0r[:, :, IPC * c + p // F, :]
    return hp


def _build(nb: int):
    ng = nb // GB

    nc = bacc.Bacc("TRN2", num_devices=8)
    xp = nc.declare_dram_parameter("xp", [ng, 2, F0, FREE], bf16, isOutput=False)
    bpp = nc.declare_dram_parameter("bp", [ng, 128, FREE], bf16, isOutput=False)
    hpp = nc.declare_dram_parameter("hp", [ng, 128, HOSTC, FREE], bf16,
                                    isOutput=False)
    kp = nc.declare_dram_parameter("kp", [128, NCHUNK, M], bf16, isOutput=False)
    rep = nc.declare_dram_parameter("rep", [104, PEC // 2, 2, 128], bf16,
                                    isOutput=False)
    outp = nc.declare_dram_parameter("out", [M, nb, D], bf16, isOutput=True)

    with ExitStack() as ctx:
        tc = ctx.enter_context(tile.TileContext(nc))
        singles = ctx.enter_context(tc.tile_pool(name="singles", bufs=1))
        xx_pool = ctx.enter_context(tc.tile_pool(name="xx", bufs=4))
        bb_pool = ctx.enter_context(tc.tile_pool(name="bb", bufs=4))
        zin_pool = ctx.enter_context(tc.tile_pool(name="zin", bufs=4))
        zc_pool = ctx.enter_context(tc.tile_pool(name="zc", bufs=3))
        osb_pool = ctx.enter_context(tc.tile_pool(name="osb", bufs=3))
        psa_pool = ctx.enter_context(tc.tile_pool(name="psa", bufs=2, space="PSUM"))
        pso_pool = ctx.enter_context(tc.tile_pool(name="pso", bufs=3, space="PSUM"))

        kw = singles.tile([128, NCHUNK, M], bf16)
        nc.sync.dma_start(out=kw, in_=kp[:])
        rp = singles.tile([104, PEC // 2, 2, 128], bf16)
        nc.sync.dma_start(out=rp, in_=rep[:])

        # HAM warm-up spin: dense back-to-back matmuls raise the PE
        # clock-gate toward 2.4 GHz while the first groups' DMAs land.
        spin_w = singles.tile([128, 128], bf16)
        nc.vector.memset(spin_w, 0.0)
        spin_r = singles.tile([128, FREE], bf16)
        nc.vector.memset(spin_r, 0.0)
        ps_spin = pso_pool.tile([128, FREE], f32, tag="pso")
        for _ in range(12):
            nc.tensor.matmul(ps_spin, spin_w, spin_r, start=True, stop=True)

        xxs = [None] * ng
        bbs = [None] * ng
        zins = [None] * ng

        def load(g):
            xx = xx_pool.tile([128, FREE], bf16, tag="xx")
            nc.sync.dma_start(out=xx[0:F0, :], in_=xp[g, 0])
            if PACK_REPS:
                nc.sync.dma_start(out=xx[64:64 + F0, :], in_=xp[g, 1])
            bb = bb_pool.tile([128, FREE], bf16, tag="bb")
            nc.sync.dma_start(out=bb, in_=bpp[g])
            zin = zin_pool.tile([128, NCHUNK, FREE], bf16, tag="zin")
            nc.sync.dma_start(out=zin[:, 0:HOSTC, :], in_=hpp[g])
            xxs[g], bbs[g], zins[g] = xx, bb, zin

        def reps(g):
            xx, zin = xxs[g], zins[g]
            for q in range(PEC // 2):
                psa = psa_pool.tile([128, 2, FREE], f32, tag="psa")
                nc.tensor.matmul(psa[:, 0, :], rp[0:F0, q, 0, :],
                                 xx[0:F0, :], start=True, stop=True)
                if PACK_REPS:
                    nc.tensor.matmul(psa[:, 1, :], rp[64:64 + F0, q, 1, :],
                                     xx[64:64 + F0, :], start=True, stop=True)
                else:
                    nc.tensor.matmul(psa[:, 1, :], rp[0:F0, q, 1, :],
                                     xx[0:F0, :], start=True, stop=True)
                c = HOSTC + 2 * q
                nc.scalar.copy(out=zin[:, c:c + 2, :], in_=psa)

        def mult(g):
            zc = zc_pool.tile([128, NCHUNK, FREE], bf16, tag="zc")
            nc.vector.tensor_tensor(
                out=zc, in0=zins[g],
                in1=bbs[g].unsqueeze(1).broadcast_to((128, NCHUNK, FREE)),
                op=mybir.AluOpType.mult)
            return zc

        zcs = [None] * ng

        def mains(g):
            zc = zcs[g]
            pso = pso_pool.tile([128, FREE], f32, tag="pso")
            for c in range(NCHUNK):
                nc.tensor.matmul(pso, kw[:, c, :], zc[:, c, :],
                                 start=(c == 0), stop=(c == NCHUNK - 1))
            osb = osb_pool.tile([128, GB, D], bf16, tag="osb")
            nc.scalar.activation(osb.rearrange("m b d -> m (b d)"), pso,
                                 mybir.ActivationFunctionType.Sigmoid)
            # out-DMA on the (otherwise idle) gpsimd queue: keeps the sync
            # queue's prefetch loads free of head-of-line blocking behind
            # sigmoid-dependent stores.
            nc.gpsimd.dma_start(out=outp[:, g * GB:(g + 1) * GB, :], in_=osb)

        for g in range(min(LOOKAHEAD + 1, ng)):
            load(g)
        for g in range(min(LOOKAHEAD, ng)):
            reps(g)
        for g in range(ng):
            if g + LOOKAHEAD + 1 < ng:
                load(g + LOOKAHEAD + 1)
            if g + LOOKAHEAD < ng:
                reps(g + LOOKAHEAD)
            zcs[g] = mult(g)
            mains(g)

    nc.finalize()
    return nc


_NC_CACHE = {}


def _get_nc():
    if "nc" not in _NC_CACHE:
        _NC_CACHE["nc"] = _build(NB)
    return _NC_CACHE["nc"]


def _make_in_maps(x0: np.ndarray, x: np.ndarray, kernel: np.ndarray):
    x0 = np.ascontiguousarray(np.asarray(x0, dtype=np.float32))
    x = np.ascontiguousarray(np.asarray(x, dtype=np.float32))
    kw = _pack_kernel(np.asarray(kernel, dtype=np.float32))
    rp = _pack_reps()
    xp = _pack_x(x0)
    bp = _pack_b(x)
    hp = _pack_hostrep(x0)
    return [
        {"xp": xp[i], "bp": bp[i], "hp": hp[i], "kp": kw, "rep": rp}
        for i in range(NCORES)
    ]


def kernel(x0: np.ndarray, x: np.ndarray, kernel: np.ndarray) -> np.ndarray:
    nc = _get_nc()
    in_maps = _make_in_maps(x0, x, kernel)
    res = run_bass_kernel_spmd(nc, in_maps, list(range(NCORES)))
    outs = [
        np.asarray(r["out"]).astype(np.float32).transpose(1, 0, 2)
        for r in res.results
    ]
    return np.ascontiguousarray(np.concatenate(outs, axis=0))


# revision 13
# speedup vs baseline: 2.5811x; 1.0086x over previous
"""Trainium2 Bass kernel for CIN: out[b,m,d] = sigmoid(einsum('bid,bjd,ijm', x0, x, K)).

Shapes (hardcoded): x0,x [4096, 40, 64] f32, kernel [40, 40, 128] f32,
out [4096, 128, 64] f32.

Sharding: data-parallel over batch B across 8 NeuronCores (512 b each).

Per-core pipeline (groups of 8 b's; free dim = 8*64 = 512), bf16 on the
engines, fp32 accumulation in PSUM.  The interaction tensor
Z[(i j), (b d)] = x0[i,(b d)] * x[j,(b d)] is built with (i j) on
partitions, blocked 3 i-rows per 128-partition chunk (14 chunks):

  - zin[p, c, bd] = x0T[3c + p//40, bd] (the replicated-x0 "A side"):
      * chunks 0..7  are HOST-replicated and DMA'd straight into zin
        (pure layout transform; DMA has headroom, PE/ACT do not)
      * chunks 8..13 come from 6 replication matmuls with constant 0/1
        weights (PSUM), evacuated to zin by the Scalar engine in 3
        pair-copies (PSUM -> SBUF bf16)
  - ONE giant DVE multiply per group builds all 14 chunks of
    zc = zin * bb:  in1 = bb[128, 512] broadcast over the chunk axis.
    All operands SBUF bf16 with unit inner stride -> DVE 2x mode
    (measured 3.88us for FD=7168).
  - 14 accumulated matmuls  pso += K_c^T @ zc_c  (contraction (i j))
  - sigmoid fused into PSUM evacuation on ACT -> bf16, DMA out as
    [M, b, d]; host transposes back to [b, M, d] and widens to f32.

Issue order is software-pipelined with lookahead 2 (reps for group g+2
issue before mains of group g) so the PE never sits behind the
DVE/ACT/DMA chain of the current group.

Host-side prep (not on the HW critical path): inputs cast to bf16 and
packed so every DMA is a dense, partition-contiguous load.
"""

import sys

for _p in ("/opt/trn_rl_repo", "/root/.axon_site/_ro/trn_rl_repo"):
    if _p not in sys.path:
        sys.path.insert(0, _p)

from contextlib import ExitStack

import numpy as np
import ml_dtypes

import concourse.bass as bass
from concourse import bacc
import concourse.tile as tile
from concourse import mybir
from concourse.bass_utils import run_bass_kernel_spmd

B, F0, F, D, M = 4096, 40, 40, 64, 128
NCORES = 8
NB = B // NCORES            # 512 b per core
GB = 8                      # b's per group
FREE = GB * D               # 512 = matmul free dim = one PSUM bank (f32)
NG = NB // GB               # 64 groups per core
IPC = 3                     # i-rows per chunk
ROWS = IPC * F              # 120 valid rows per chunk
NCHUNK = (F0 + IPC - 1) // IPC  # 14
HOSTC = 8                   # chunks 0..HOSTC-1 replicated on the host
PEC = NCHUNK - HOSTC        # 6 chunks replicated on the PE (3 pairs)
PACK_REPS = True            # tile_position row-packing of rep pairs
LOOKAHEAD = 2

f32 = mybir.dt.float32
bf16 = mybir.dt.bfloat16
BF16 = ml_dtypes.bfloat16


def _pack_kernel(kernel_np: np.ndarray) -> np.ndarray:
    """K[i,j,m] -> kwT [128, NCHUNK, M] bf16,
    kwT[p, c, m] = K[3c + p//40, p%40, m] (zero where invalid)."""
    kf = np.zeros((NCHUNK, 128, M), dtype=np.float32)
    p = np.arange(ROWS)
    for c in range(NCHUNK):
        i = IPC * c + p // F
        valid = i < F0
        kf[c, p[valid]] = kernel_np[i[valid], p[valid] % F]
    return np.ascontiguousarray(kf.transpose(1, 0, 2).astype(BF16))


def _pack_reps() -> np.ndarray:
    """Constant replication weights [104, PEC//2, 2, 128] bf16 for the PE
    chunks (HOSTC..NCHUNK-1).  Slot [0:40, q, s] holds the weights for
    chunk HOSTC+2q+s (base-0 operands, unpacked mode); slot
    [64:104, q, 1] duplicates the odd chunk's weights so a packed pair
    can run as row-tiles (0,0) and (64,0)."""
    rp = np.zeros((104, PEC // 2, 2, 128), dtype=np.float32)
    p = np.arange(ROWS)
    for q in range(PEC // 2):
        for s in (0, 1):
            c = HOSTC + 2 * q + s
            i = IPC * c + p // F
            valid = i < F0
            rp[i[valid], q, s, p[valid]] = 1.0
            if s == 1:
                rp[64 + i[valid], q, s, p[valid]] = 1.0
    return np.ascontiguousarray(rp.astype(BF16))


def _pack_x(x0: np.ndarray) -> np.ndarray:
    """-> xp [NCORES, NG, 2, F0, FREE] bf16: x0T per (core, group),
    duplicated so a copy can sit at partitions 64:104 for packed reps."""
    x0r = x0.reshape(NCORES, NG, GB, F0, D).transpose(0, 1, 3, 2, 4)
    x0r = x0r.reshape(NCORES, NG, F0, FREE).astype(BF16)
    return np.ascontiguousarray(
        np.broadcast_to(x0r[:, :, None], (NCORES, NG, 2, F0, FREE)))


def _pack_b(x: np.ndarray) -> np.ndarray:
    """-> bp [NCORES, NG, 128, FREE] bf16: B[p, bd] = xT[p%40, bd] for
    p < 120, zero pad rows."""
    xr = x.reshape(NCORES, NG, GB, F, D).transpose(0, 1, 3, 2, 4)
    xr = xr.reshape(NCORES, NG, F, FREE).astype(BF16)
    bp = np.zeros((NCORES, NG, 128, FREE), dtype=BF16)
    bp[:, :, 0:ROWS, :] = np.concatenate([xr] * IPC, axis=2)
    return bp


def _pack_hostrep(x0: np.ndarray) -> np.ndarray:
    """-> hp [NCORES, NG, 128, HOSTC, FREE] bf16:
    hp[.., p, c, bd] = x0T[3c + p//40, bd] for p < 120, zero pad rows."""
    x0r = x0.reshape(NCORES, NG, GB, F0, D).transpose(0, 1, 3, 2, 4)
    x0r = np.ascontiguousarray(x0r.reshape(NCORES, NG, F0, FREE)).astype(BF16)
    hp = np.zeros((NCORES, NG, 128, HOSTC, FREE), dtype=BF16)
    p = np.arange(ROWS)
    for c in range(HOSTC):
        hp[:, :, 0:ROWS, c, :] = x0r[:, :, IPC * c + p // F, :]
    return hp


def _build(nb: int):
    ng = nb // GB

    nc = bacc.Bacc("TRN2", num_devices=8)
    xp = nc.declare_dram_parameter("xp", [ng, 2, F0, FREE], bf16, isOutput=False)
    bpp = nc.declare_dram_parameter("bp", [ng, 128, FREE], bf16, isOutput=False)
    hpp = nc.declare_dram_parameter("hp", [ng, 128, HOSTC, FREE], bf16,
                                    isOutput=False)
    kp = nc.declare_dram_parameter("kp", [128, NCHUNK, M], bf16, isOutput=False)
    rep = nc.declare_dram_parameter("rep", [104, PEC // 2, 2, 128], bf16,
                                    isOutput=False)
    outp = nc.declare_dram_parameter("out", [M, nb, D], bf16, isOutput=True)

    with ExitStack() as ctx:
        tc = ctx.enter_context(tile.TileContext(nc))
        singles = ctx.enter_context(tc.tile_pool(name="singles", bufs=1))
        xx_pool = ctx.enter_context(tc.tile_pool(name="xx", bufs=5))
        bb_pool = ctx.enter_context(tc.tile_pool(name="bb", bufs=5))
        zin_pool = ctx.enter_context(tc.tile_pool(name="zin", bufs=5))
        zc_pool = ctx.enter_context(tc.tile_pool(name="zc", bufs=3))
        osb_pool = ctx.enter_context(tc.tile_pool(name="osb", bufs=3))
        psa_pool = ctx.enter_context(tc.tile_pool(name="psa", bufs=3, space="PSUM"))
        pso_pool = ctx.enter_context(tc.tile_pool(name="pso", bufs=2, space="PSUM"))

        kw = singles.tile([128, NCHUNK, M], bf16)
        nc.sync.dma_start(out=kw, in_=kp[:])
        rp = singles.tile([104, PEC // 2, 2, 128], bf16)
        nc.sync.dma_start(out=rp, in_=rep[:])

        # HAM warm-up spin: dense back-to-back matmuls raise the PE
        # clock-gate toward 2.4 GHz while the first groups' DMAs land.
        spin_w = singles.tile([128, 128], bf16)
        nc.vector.memset(spin_w, 0.0)
        spin_r = singles.tile([128, FREE], bf16)
        nc.vector.memset(spin_r, 0.0)
        ps_spin = pso_pool.tile([128, FREE], f32, tag="pso")
        for _ in range(12):
            nc.tensor.matmul(ps_spin, spin_w, spin_r, start=True, stop=True)

        xxs = [None] * ng
        bbs = [None] * ng
        zins = [None] * ng

        def load(g):
            xx = xx_pool.tile([128, FREE], bf16, tag="xx")
            nc.sync.dma_start(out=xx[0:F0, :], in_=xp[g, 0])
            if PACK_REPS:
                nc.sync.dma_start(out=xx[64:64 + F0, :], in_=xp[g, 1])
            bb = bb_pool.tile([128, FREE], bf16, tag="bb")
            nc.sync.dma_start(out=bb, in_=bpp[g])
            zin = zin_pool.tile([128, NCHUNK, FREE], bf16, tag="zin")
            nc.sync.dma_start(out=zin[:, 0:HOSTC, :], in_=hpp[g])
            xxs[g], bbs[g], zins[g] = xx, bb, zin

        def reps(g):
            xx, zin = xxs[g], zins[g]
            for q in range(PEC // 2):
                psa = psa_pool.tile([128, 2, FREE], f32, tag="psa")
                nc.tensor.matmul(psa[:, 0, :], rp[0:F0, q, 0, :],
                                 xx[0:F0, :], start=True, stop=True)
                if PACK_REPS:
                    nc.tensor.matmul(psa[:, 1, :], rp[64:64 + F0, q, 1, :],
                                     xx[64:64 + F0, :], start=True, stop=True)
                else:
                    nc.tensor.matmul(psa[:, 1, :], rp[0:F0, q, 1, :],
                                     xx[0:F0, :], start=True, stop=True)
                c = HOSTC + 2 * q
                nc.scalar.copy(out=zin[:, c:c + 2, :], in_=psa)

        def mult(g):
            zc = zc_pool.tile([128, NCHUNK, FREE], bf16, tag="zc")
            nc.vector.tensor_tensor(
                out=zc, in0=zins[g],
                in1=bbs[g].unsqueeze(1).broadcast_to((128, NCHUNK, FREE)),
                op=mybir.AluOpType.mult)
            return zc

        zcs = [None] * ng

        def mains(g):
            zc = zcs[g]
            pso = pso_pool.tile([128, FREE], f32, tag="pso")
            for c in range(NCHUNK):
                nc.tensor.matmul(pso, kw[:, c, :], zc[:, c, :],
                                 start=(c == 0), stop=(c == NCHUNK - 1))
            osb = osb_pool.tile([128, GB, D], bf16, tag="osb")
            nc.scalar.activation(osb.rearrange("m b d -> m (b d)"), pso,
                                 mybir.ActivationFunctionType.Sigmoid)
            # out-DMA on the (otherwise idle) gpsimd queue: keeps the sync
            # queue's prefetch loads free of head-of-line blocking behind
            # sigmoid-dependent stores.
            nc.gpsimd.dma_start(out=outp[:, g * GB:(g + 1) * GB, :], in_=osb)

        for g in range(min(LOOKAHEAD + 1, ng)):
            load(g)
        for g in range(min(LOOKAHEAD, ng)):
            reps(g)
        for g in range(ng):
            if g + LOOKAHEAD + 1 < ng:
                load(g + LOOKAHEAD + 1)
            zcs[g] = mult(g)
            mains(g)
            # reps (and their ACT evacs) AFTER mains(g): the sigmoid(g)
            # lands on the ACT queue ahead of the evacs(g+2), so mains(g+2)
            # is never blocked behind a late sigmoid via the pso pool.
            if g + LOOKAHEAD < ng:
                reps(g + LOOKAHEAD)

    nc.finalize()
    return nc


_NC_CACHE = {}


def _get_nc():
    if "nc" not in _NC_CACHE:
        _NC_CACHE["nc"] = _build(NB)
    return _NC_CACHE["nc"]


def _make_in_maps(x0: np.ndarray, x: np.ndarray, kernel: np.ndarray):
    x0 = np.ascontiguousarray(np.asarray(x0, dtype=np.float32))
    x = np.ascontiguousarray(np.asarray(x, dtype=np.float32))
    kw = _pack_kernel(np.asarray(kernel, dtype=np.float32))
    rp = _pack_reps()
    xp = _pack_x(x0)
    bp = _pack_b(x)
    hp = _pack_hostrep(x0)
    return [
        {"xp": xp[i], "bp": bp[i], "hp": hp[i], "kp": kw, "rep": rp}
        for i in range(NCORES)
    ]


def kernel(x0: np.ndarray, x: np.ndarray, kernel: np.ndarray) -> np.ndarray:
    nc = _get_nc()
    in_maps = _make_in_maps(x0, x, kernel)
    res = run_bass_kernel_spmd(nc, in_maps, list(range(NCORES)))
    outs = [
        np.asarray(r["out"]).astype(np.float32).transpose(1, 0, 2)
        for r in res.results
    ]
    return np.ascontiguousarray(np.concatenate(outs, axis=0))


# revision 15
# speedup vs baseline: 2.6473x; 1.0256x over previous
"""Trainium2 Bass kernel for CIN: out[b,m,d] = sigmoid(einsum('bid,bjd,ijm', x0, x, K)).

Shapes (hardcoded): x0,x [4096, 40, 64] f32, kernel [40, 40, 128] f32,
out [4096, 128, 64] f32.

Sharding: data-parallel over batch B across 8 NeuronCores (512 b each).

Per-core pipeline (groups of 8 b's; free dim = 8*64 = 512), bf16 on the
engines, fp32 accumulation in PSUM.  The interaction tensor
Z[(i j), (b d)] = x0[i,(b d)] * x[j,(b d)] is built with (i j) on
partitions, blocked 3 i-rows per 128-partition chunk (14 chunks):

  - zin[p, c, bd] = x0T[3c + p//40, bd] (the replicated-x0 "A side"):
      * chunks 0..7  are HOST-replicated and DMA'd straight into zin
        (pure layout transform; DMA has headroom, PE/ACT do not)
      * chunks 8..13 come from 6 replication matmuls with constant 0/1
        weights (PSUM), evacuated to zin by the Scalar engine in 3
        pair-copies (PSUM -> SBUF bf16)
  - ONE giant DVE multiply per group builds all 14 chunks of
    zc = zin * bb:  in1 = bb[128, 512] broadcast over the chunk axis.
    All operands SBUF bf16 with unit inner stride -> DVE 2x mode
    (measured 3.88us for FD=7168).
  - 14 accumulated matmuls  pso += K_c^T @ zc_c  (contraction (i j))
  - sigmoid fused into PSUM evacuation on ACT -> bf16, DMA out as
    [M, b, d]; host transposes back to [b, M, d] and widens to f32.

Issue order is software-pipelined with lookahead 3: loads prefetch 4
groups ahead (absorbs HBM jitter; aggregate in-stream is ~320 GB/s per
core), replication matmuls + evacs run 3 groups ahead, and the sigmoid
of group g is issued to the Scalar queue before the evacs of group g+3
so the pso pool never blocks the next mains.  Steady state is DVE-bound
at ~4.3us per group (giant TT 3.89us + handoff); PE ~3.4us, ACT ~4.0us,
with the packed replication pairs keeping the PE warm (HAM at 2.4 GHz).

Host-side prep (not on the HW critical path): inputs cast to bf16 and
packed so every DMA is a dense, partition-contiguous load.
"""

import sys

for _p in ("/opt/trn_rl_repo", "/root/.axon_site/_ro/trn_rl_repo"):
    if _p not in sys.path:
        sys.path.insert(0, _p)

from contextlib import ExitStack

import numpy as np
import ml_dtypes

import concourse.bass as bass
from concourse import bacc
import concourse.tile as tile
from concourse import mybir
from concourse.bass_utils import run_bass_kernel_spmd

B, F0, F, D, M = 4096, 40, 40, 64, 128
NCORES = 8
NB = B // NCORES            # 512 b per core
GB = 8                      # b's per group
FREE = GB * D               # 512 = matmul free dim = one PSUM bank (f32)
NG = NB // GB               # 64 groups per core
IPC = 3                     # i-rows per chunk
ROWS = IPC * F              # 120 valid rows per chunk
NCHUNK = (F0 + IPC - 1) // IPC  # 14
HOSTC = 8                   # chunks 0..HOSTC-1 replicated on the host
PEC = NCHUNK - HOSTC        # 6 chunks replicated on the PE (3 pairs)
PACK_REPS = True            # tile_position row-packing of rep pairs
LOOKAHEAD = 3

f32 = mybir.dt.float32
bf16 = mybir.dt.bfloat16
BF16 = ml_dtypes.bfloat16


def _pack_kernel(kernel_np: np.ndarray) -> np.ndarray:
    """K[i,j,m] -> kwT [128, NCHUNK, M] bf16,
    kwT[p, c, m] = K[3c + p//40, p%40, m] (zero where invalid)."""
    kf = np.zeros((NCHUNK, 128, M), dtype=np.float32)
    p = np.arange(ROWS)
    for c in range(NCHUNK):
        i = IPC * c + p // F
        valid = i < F0
        kf[c, p[valid]] = kernel_np[i[valid], p[valid] % F]
    return np.ascontiguousarray(kf.transpose(1, 0, 2).astype(BF16))


def _pack_reps() -> np.ndarray:
    """Constant replication weights [104, PEC//2, 2, 128] bf16 for the PE
    chunks (HOSTC..NCHUNK-1).  Slot [0:40, q, s] holds the weights for
    chunk HOSTC+2q+s (base-0 operands, unpacked mode); slot
    [64:104, q, 1] duplicates the odd chunk's weights so a packed pair
    can run as row-tiles (0,0) and (64,0)."""
    rp = np.zeros((104, PEC // 2, 2, 128), dtype=np.float32)
    p = np.arange(ROWS)
    for q in range(PEC // 2):
        for s in (0, 1):
            c = HOSTC + 2 * q + s
            i = IPC * c + p // F
            valid = i < F0
            rp[i[valid], q, s, p[valid]] = 1.0
            if s == 1:
                rp[64 + i[valid], q, s, p[valid]] = 1.0
    return np.ascontiguousarray(rp.astype(BF16))


def _pack_x(x0: np.ndarray) -> np.ndarray:
    """-> xp [NCORES, NG, 2, F0, FREE] bf16: x0T per (core, group),
    duplicated so a copy can sit at partitions 64:104 for packed reps."""
    x0r = x0.reshape(NCORES, NG, GB, F0, D).transpose(0, 1, 3, 2, 4)
    x0r = x0r.reshape(NCORES, NG, F0, FREE).astype(BF16)
    return np.ascontiguousarray(
        np.broadcast_to(x0r[:, :, None], (NCORES, NG, 2, F0, FREE)))


def _pack_b(x: np.ndarray) -> np.ndarray:
    """-> bp [NCORES, NG, 128, FREE] bf16: B[p, bd] = xT[p%40, bd] for
    p < 120, zero pad rows."""
    xr = x.reshape(NCORES, NG, GB, F, D).transpose(0, 1, 3, 2, 4)
    xr = xr.reshape(NCORES, NG, F, FREE).astype(BF16)
    bp = np.zeros((NCORES, NG, 128, FREE), dtype=BF16)
    bp[:, :, 0:ROWS, :] = np.concatenate([xr] * IPC, axis=2)
    return bp


def _pack_hostrep(x0: np.ndarray) -> np.ndarray:
    """-> hp [NCORES, NG, 128, HOSTC, FREE] bf16:
    hp[.., p, c, bd] = x0T[3c + p//40, bd] for p < 120, zero pad rows."""
    x0r = x0.reshape(NCORES, NG, GB, F0, D).transpose(0, 1, 3, 2, 4)
    x0r = np.ascontiguousarray(x0r.reshape(NCORES, NG, F0, FREE)).astype(BF16)
    hp = np.zeros((NCORES, NG, 128, HOSTC, FREE), dtype=BF16)
    p = np.arange(ROWS)
    for c in range(HOSTC):
        hp[:, :, 0:ROWS, c, :] = x0r[:, :, IPC * c + p // F, :]
    return hp


def _build(nb: int):
    ng = nb // GB

    nc = bacc.Bacc("TRN2", num_devices=8)
    xp = nc.declare_dram_parameter("xp", [ng, 2, F0, FREE], bf16, isOutput=False)
    bpp = nc.declare_dram_parameter("bp", [ng, 128, FREE], bf16, isOutput=False)
    hpp = nc.declare_dram_parameter("hp", [ng, 128, HOSTC, FREE], bf16,
                                    isOutput=False)
    kp = nc.declare_dram_parameter("kp", [128, NCHUNK, M], bf16, isOutput=False)
    rep = nc.declare_dram_parameter("rep", [104, PEC // 2, 2, 128], bf16,
                                    isOutput=False)
    outp = nc.declare_dram_parameter("out", [M, nb, D], bf16, isOutput=True)

    with ExitStack() as ctx:
        tc = ctx.enter_context(tile.TileContext(nc))
        singles = ctx.enter_context(tc.tile_pool(name="singles", bufs=1))
        xx_pool = ctx.enter_context(tc.tile_pool(name="xx", bufs=6))
        bb_pool = ctx.enter_context(tc.tile_pool(name="bb", bufs=6))
        zin_pool = ctx.enter_context(tc.tile_pool(name="zin", bufs=6))
        zc_pool = ctx.enter_context(tc.tile_pool(name="zc", bufs=3))
        osb_pool = ctx.enter_context(tc.tile_pool(name="osb", bufs=3))
        psa_pool = ctx.enter_context(tc.tile_pool(name="psa", bufs=3, space="PSUM"))
        pso_pool = ctx.enter_context(tc.tile_pool(name="pso", bufs=2, space="PSUM"))

        kw = singles.tile([128, NCHUNK, M], bf16)
        nc.sync.dma_start(out=kw, in_=kp[:])
        rp = singles.tile([104, PEC // 2, 2, 128], bf16)
        nc.sync.dma_start(out=rp, in_=rep[:])

        # HAM warm-up spin: dense back-to-back matmuls raise the PE
        # clock-gate toward 2.4 GHz while the first groups' DMAs land.
        spin_w = singles.tile([128, 128], bf16)
        nc.vector.memset(spin_w, 0.0)
        spin_r = singles.tile([128, FREE], bf16)
        nc.vector.memset(spin_r, 0.0)
        ps_spin = pso_pool.tile([128, FREE], f32, tag="pso")
        for _ in range(12):
            nc.tensor.matmul(ps_spin, spin_w, spin_r, start=True, stop=True)

        xxs = [None] * ng
        bbs = [None] * ng
        zins = [None] * ng

        def load(g):
            xx = xx_pool.tile([128, FREE], bf16, tag="xx")
            nc.sync.dma_start(out=xx[0:F0, :], in_=xp[g, 0])
            if PACK_REPS:
                nc.sync.dma_start(out=xx[64:64 + F0, :], in_=xp[g, 1])
            bb = bb_pool.tile([128, FREE], bf16, tag="bb")
            nc.sync.dma_start(out=bb, in_=bpp[g])
            zin = zin_pool.tile([128, NCHUNK, FREE], bf16, tag="zin")
            nc.sync.dma_start(out=zin[:, 0:HOSTC, :], in_=hpp[g])
            xxs[g], bbs[g], zins[g] = xx, bb, zin

        def reps(g):
            xx, zin = xxs[g], zins[g]
            for q in range(PEC // 2):
                psa = psa_pool.tile([128, 2, FREE], f32, tag="psa")
                nc.tensor.matmul(psa[:, 0, :], rp[0:F0, q, 0, :],
                                 xx[0:F0, :], start=True, stop=True)
                if PACK_REPS:
                    nc.tensor.matmul(psa[:, 1, :], rp[64:64 + F0, q, 1, :],
                                     xx[64:64 + F0, :], start=True, stop=True)
                else:
                    nc.tensor.matmul(psa[:, 1, :], rp[0:F0, q, 1, :],
                                     xx[0:F0, :], start=True, stop=True)
                c = HOSTC + 2 * q
                nc.scalar.copy(out=zin[:, c:c + 2, :], in_=psa)

        def mult(g):
            zc = zc_pool.tile([128, NCHUNK, FREE], bf16, tag="zc")
            nc.vector.tensor_tensor(
                out=zc, in0=zins[g],
                in1=bbs[g].unsqueeze(1).broadcast_to((128, NCHUNK, FREE)),
                op=mybir.AluOpType.mult)
            return zc

        zcs = [None] * ng

        def mains(g):
            zc = zcs[g]
            pso = pso_pool.tile([128, FREE], f32, tag="pso")
            for c in range(NCHUNK):
                nc.tensor.matmul(pso, kw[:, c, :], zc[:, c, :],
                                 start=(c == 0), stop=(c == NCHUNK - 1))
            osb = osb_pool.tile([128, GB, D], bf16, tag="osb")
            nc.scalar.activation(osb.rearrange("m b d -> m (b d)"), pso,
                                 mybir.ActivationFunctionType.Sigmoid)
            # out-DMA on the (otherwise idle) gpsimd queue: keeps the sync
            # queue's prefetch loads free of head-of-line blocking behind
            # sigmoid-dependent stores.
            nc.gpsimd.dma_start(out=outp[:, g * GB:(g + 1) * GB, :], in_=osb)

        for g in range(min(LOOKAHEAD + 1, ng)):
            load(g)
        for g in range(min(LOOKAHEAD, ng)):
            reps(g)
        for g in range(ng):
            if g + LOOKAHEAD + 1 < ng:
                load(g + LOOKAHEAD + 1)
            zcs[g] = mult(g)
            mains(g)
            # reps (and their ACT evacs) AFTER mains(g): the sigmoid(g)
            # lands on the ACT queue ahead of the evacs(g+2), so mains(g+2)
            # is never blocked behind a late sigmoid via the pso pool.
            if g + LOOKAHEAD < ng:
                reps(g + LOOKAHEAD)

    nc.finalize()
    return nc


_NC_CACHE = {}


def _get_nc():
    if "nc" not in _NC_CACHE:
        _NC_CACHE["nc"] = _build(NB)
    return _NC_CACHE["nc"]


def _make_in_maps(x0: np.ndarray, x: np.ndarray, kernel: np.ndarray):
    x0 = np.ascontiguousarray(np.asarray(x0, dtype=np.float32))
    x = np.ascontiguousarray(np.asarray(x, dtype=np.float32))
    kw = _pack_kernel(np.asarray(kernel, dtype=np.float32))
    rp = _pack_reps()
    xp = _pack_x(x0)
    bp = _pack_b(x)
    hp = _pack_hostrep(x0)
    return [
        {"xp": xp[i], "bp": bp[i], "hp": hp[i], "kp": kw, "rep": rp}
        for i in range(NCORES)
    ]


def kernel(x0: np.ndarray, x: np.ndarray, kernel: np.ndarray) -> np.ndarray:
    nc = _get_nc()
    in_maps = _make_in_maps(x0, x, kernel)
    res = run_bass_kernel_spmd(nc, in_maps, list(range(NCORES)))
    outs = [
        np.asarray(r["out"]).astype(np.float32).transpose(1, 0, 2)
        for r in res.results
    ]
    return np.ascontiguousarray(np.concatenate(outs, axis=0))
